# revision 1
# baseline (speedup 1.0000x reference)
"""Decoder-block Bass kernel, v2: fp8 DoubleRow matmul paths.

Per-core program (SPMD over 8 cores = 4 batches x 2 query-halves):
  T=2048 self keys, TQ=1024 own queries, MC=1152 compacted cross keys.
Layout: transposed activations [feature, token]; activations quantized to
fp8e4m3 x16 in feature-pair tiles [128, 2, cols]; weights fp8 x4096 with
optional same-scale residual term (weight compensation); PSUM descale 2^-16.
Role 0 places its valid self-keys in blocks 8..15 (blocks 0..7 bias-masked)
so causal stair slots coincide with role 1 and the program needs only 4
stair-mask multiplies per head.
"""
import sys
sys.path.insert(0, '/opt/trn_rl_repo')
import concourse.bass as bass
import concourse.tile as tile
from concourse import mybir

F32 = mybir.dt.float32
BF16 = mybir.dt.bfloat16
FP8 = mybir.dt.float8e4
AF = mybir.ActivationFunctionType
ALU = mybir.AluOpType
DR = mybir.MatmulPerfMode.DoubleRow

D, H, DK, T, M, DFF, TQ = 1024, 16, 64, 2048, 1152, 4096, 1024
ND, NKB = D // 128, T // 128
ND2 = ND // 2
NKB2 = NKB // 2
NKC = M // 128          # 9 cross key blocks
NKC2 = NKC // 2         # 4 pairs + 1 single
NF = DFF // 128
NF2 = NF // 2

WS = 2048.0             # weight fp8 scale (e4m3 max finite = 240)
XS = 16.0               # activation fp8 scale
DS = 1.0 / (WS * XS)    # psum descale
VS = 16.0               # V storage scale (Vt = VS * v)

# per-layer precision: 'bf' | 'f8' | 'f8w'
PREC = {
    'qkv': 'f8w', 'sap': 'f8w', 'caq': 'f8', 'cakv': 'f8',
    'cap': 'f8', 'ff1': 'f8w', 'ff2': 'f8w',
}

# ---- tile drain walrus workaround -------------------------------------------
from concourse.vector_clock import ScopedClock
def _drain_and_barrier(self, tick_clock, wait_clock):
    nops = [self.nc.sync.nop(nofuse=True, hint=f"drain_split_{i}").ins
            for i in range(32)]
    drain_inst = self.nc.sync.drain()
    wait_clock.add_sem_waits(drain_inst.ins,
                             ScopedClock({None: tick_clock.global_clock}))
    di = drain_inst.ins
    si = di.sync_info
    waits = list(si.on_wait) if si is not None and si.on_wait else []
    if len(waits) > 1:
        for i, w in enumerate(waits[:-1]):
            ni = nops[i]
            nsi = ni.sync_info
            if nsi is None:
                ni.sync_info = mybir.SyncInfo(on_wait=[w], on_update=[])
            else:
                ow = list(nsi.on_wait) if nsi.on_wait else []
                ow.append(w)
                nsi.on_wait = ow
        si.on_wait = waits[-1:]
    self.nc.all_engine_barrier()
    assert self.sems is not None
    popped = self.nc._tile_sem_poison_stack.pop()
    assert popped is self._sem_poison
    self.nc.clear_and_free_semaphores(list(self.sems.allocated().values()))
    self.nc.all_engine_barrier()
tile.TileContext._drain_and_barrier = _drain_and_barrier
# -----------------------------------------------------------------------------

_fix_ctr = [0]

def fixup_waits(nc, maxw=1):
    """walrus build rejects >~2 sync waits per instruction; hoist extras
    onto same-engine NOPs inserted just before."""
    for f in nc.m.functions:
        for bb in f.blocks:
            newl = []
            for inst in bb.instructions:
                si = inst.sync_info
                waits = list(si.on_wait) if si is not None and si.on_wait else []
                if len(waits) > maxw:
                    extra, keep = waits[:-maxw], waits[-maxw:]
                    for w in extra:
                        _fix_ctr[0] += 1
                        nop = mybir.InstNoOp(
                            name=f"waitfix_{_fix_ctr[0]}", ins=[], outs=[],
                            sync_info=mybir.SyncInfo(on_wait=[w], on_update=[]))
                        nop.engine = inst.engine
                        newl.append(nop)
                    si.on_wait = keep
                newl.append(inst)
            bb.instructions[:] = newl
    return nc


def nps_of(prec):
    return {'bf': 1, 'f8': 1, 'f8w': 2}[prec]


class KB:
    def __init__(self, nc, tc):
        self.nc, self.tc = nc, tc

    def src_chunk(self, spec, dt, w, strm):
        kind = spec[0]
        if kind == "sbuf":
            return spec[1][dt][:, spec[2]: spec[2] + w]
        hnd, col0 = spec[1], spec[2]
        t = strm.tile([128, w], F32, tag="lnsrc", name="lnsrc")
        self.nc.sync.dma_start(
            out=t, in_=hnd[dt * 128:(dt + 1) * 128, col0: col0 + w])
        return t

    def layernorm(self, pools, src, tcols, w_dram, b_dram, out_pairs, ocol0,
                  trivial=False):
        """out_pairs: ND2 fp8 tiles [128, 2, ...]; writes XS * ln(x)."""
        nc = self.nc
        pool_lin, pool_sp, ln_tmp, strm = pools
        ntc = tcols // 512
        if not trivial:
            w_row = ln_tmp.tile([1, D], BF16, tag="lnwrow", name="lnwrow", bufs=2)
            nc.gpsimd.dma_start(out=w_row, in_=w_dram[:, :])
            b_row = ln_tmp.tile([1, D], BF16, tag="lnbrow", name="lnbrow", bufs=2)
            nc.gpsimd.dma_start(out=b_row, in_=b_dram[:, :])
        stats = []
        for tci in range(ntc):
            st = pool_lin.tile([33, 512], F32, tag="ps", name="stats")
            stats.append(st)
        xbs = []
        for dt in range(ND):
            xb = ln_tmp.tile([128, tcols], BF16, tag=f"lnxb{dt}", name=f"lnxb{dt}",
                             bufs=1)
            if src[0] == "dram":
                hnd, col0 = src[1], src[2]
                nc.sync.dma_start(
                    out=xb, in_=hnd[dt * 128:(dt + 1) * 128, col0: col0 + tcols])
            else:
                nc.gpsimd.tensor_copy(out=xb, in_=src[1][dt][:, src[2]: src[2] + tcols])
            xbs.append(xb)
            xq = ln_tmp.tile([128, tcols], BF16, tag="lnsq", name="lnsq", bufs=2)
            nc.vector.tensor_tensor(out=xq, in0=xb, in1=xb, op=ALU.mult)
            for tci in range(ntc):
                cs = slice(tci * 512, (tci + 1) * 512)
                nc.tensor.matmul(stats[tci][0:1, :], self.ones128_bf[:, :],
                                 xb[:, cs], start=(dt == 0), stop=(dt == ND - 1))
                nc.tensor.matmul(stats[tci][32:33, :], self.ones128_bf[:, :],
                                 xq[:, cs], start=(dt == 0), stop=(dt == ND - 1))
        rows = []
        for tci in range(ntc):
            mu = ln_tmp.tile([1, 512], F32, tag="lnmu", name="lnmu", bufs=1)
            nc.scalar.mul(out=mu, in_=stats[tci][0:1, :], mul=1.0 / D)
            msq = ln_tmp.tile([1, 512], F32, tag="lnmsq", name="lnmsq", bufs=1)
            nc.scalar.mul(out=msq, in_=stats[tci][32:33, :], mul=1.0 / D)
            mu2 = ln_tmp.tile([1, 512], F32, tag="lnmu2", name="lnmu2", bufs=1)
            nc.vector.tensor_tensor(out=mu2, in0=mu, in1=mu, op=ALU.mult)
            nc.vector.tensor_tensor(out=msq, in0=msq, in1=mu2, op=ALU.subtract)
            nc.scalar.activation(out=msq, in_=msq, func=AF.Sqrt, bias=self.eps_t)
            rstd = ln_tmp.tile([1, 512], BF16, tag="lnrstd", name="lnrstd", bufs=2)
            nc.vector.reciprocal(out=rstd, in_=msq)
            # scale by XS so the fp8 output is XS * ln(x)
            nc.vector.tensor_scalar(out=rstd, in0=rstd, scalar1=XS, scalar2=None,
                                    op0=ALU.mult)
            musig = ln_tmp.tile([1, 512], BF16, tag="lnmusig", name="lnmusig", bufs=2)
            nc.vector.tensor_tensor(out=musig, in0=mu, in1=rstd, op=ALU.mult)
            rows.append((rstd, musig))
        acbs = []
        if trivial:
            for tci in range(ntc):
                rstd, musig = rows[tci]
                ac = pool_sp.tile([128, 2, 512], F32, tag="S2", name="lnac")
                nc.tensor.matmul(ac[:, 0, :], self.ones1x128, rstd[:, :],
                                 start=True, stop=True)
                nc.tensor.matmul(ac[:, 1, :], self.ones1x128, musig[:, :],
                                 start=True, stop=True)
                acb = ln_tmp.tile([128, 1024], BF16, tag="lnacb", name="lnacb",
                                  bufs=2)
                nc.vector.tensor_copy(out=acb, in_=ac.rearrange("p a b -> p (a b)"))
                acbs.append(acb)
        for dt in range(ND):
            ds_ = slice(dt * 128, (dt + 1) * 128)
            xb = xbs[dt]
            for tci in range(ntc):
                rstd, musig = rows[tci]
                if trivial:
                    acb = acbs[tci]
                else:
                    ac = pool_sp.tile([128, 2, 512], F32, tag="S2", name="lnac")
                    nc.tensor.matmul(ac[:, 0, :], w_row[:, ds_], rstd[:, :],
                                     start=True, stop=True)
                    nc.tensor.matmul(ac[:, 1, :], w_row[:, ds_], musig[:, :],
                                     start=True, stop=False)
                    nc.tensor.matmul(ac[:, 1, :], b_row[:, ds_],
                                     self.negXS[:, :], start=False, stop=True)
                    acb = ln_tmp.tile([128, 1024], BF16, tag="lnacb", name="lnacb",
                                      bufs=2)
                    nc.vector.tensor_copy(out=acb, in_=ac.rearrange("p a b -> p (a b)"))
                cs = slice(tci * 512, (tci + 1) * 512)
                tmp = ln_tmp.tile([128, 512], BF16, tag="lnapply", name="lnapply")
                nc.vector.tensor_tensor(out=tmp, in0=xb[:, cs], in1=acb[:, 0:512],
                                        op=ALU.mult)
                nc.gpsimd.tensor_tensor(
                    out=out_pairs[dt // 2][:, dt % 2,
                                           ocol0 + tci * 512: ocol0 + (tci + 1) * 512],
                    in0=tmp, in1=acb[:, 512:1024], op=ALU.subtract)

    def linear(self, pool_lin, wpool, w_dram, n_oblk, rhs_pairs, rcol0, tcols,
               consumer, nps=2, nc2=ND2, bias_row=None, strip_tag="w"):
        """fp8 DoubleRow linear. w_dram [128, n_oblk*nps*nc2*2*128] fp8.
        consumer(ob, tci, ps_view) handles PSUM -> dest (incl. descale)."""
        nc = self.nc
        ntc = (tcols + 511) // 512
        blk = nps * nc2 * 2 * 128
        for ob in range(n_oblk):
            wst = wpool.tile([128, nps, nc2, 2, 128], FP8, tag=strip_tag,
                             name=strip_tag)
            nc.sync.dma_start(
                out=wst,
                in_=w_dram[:, ob * blk:(ob + 1) * blk].rearrange(
                    "p (t c j o) -> p t c j o", t=nps, c=nc2, j=2))
            for tci in range(ntc):
                w512 = min(512, tcols - tci * 512)
                ps = pool_lin.tile([128, 512], F32, tag="ps", name="linps")
                first = True
                if bias_row is not None:
                    og = ob * 128
                    nc.tensor.matmul(ps[:, 0:w512], bias_row[:, og:og + 128],
                                     self.ones512[:, 0:w512], start=True,
                                     stop=False)
                    first = False
                for t in range(nps):
                    for c2 in range(nc2):
                        rhs = rhs_pairs[c2][:, :, rcol0 + tci * 512:
                                            rcol0 + tci * 512 + w512]
                        nc.tensor.matmul(ps[:, 0:w512], wst[:, t, c2, :, :], rhs,
                                         start=first,
                                         stop=(t == nps - 1 and c2 == nc2 - 1),
                                         perf_mode=DR)
                        first = False
                consumer(ob, tci, ps[:, 0:w512])

    def vproj(self, pool_lin, wpool, w_dram, b_row, stat_pairs, scol0, Vt2, kbs,
              nps=2, vmask_sb=None, tag="wv"):
        """V projection: out[keys, 512] = stationary(h pairs) @ moving w pairs.
        w_dram [128, 2*nps*ND2*2*512] fp8. kbs: global key block indices.
        Writes Vt2[kb//2][:, kb%2, oc*8:(oc+1)*8, 0:64] = VS * v."""
        nc = self.nc
        for oc in range(2):
            wvt = []
            for t in range(nps):
                for c2 in range(ND2):
                    idx = ((oc * nps + t) * ND2 + c2) * 1024
                    wt = wpool.tile([128, 2, 512], FP8, tag=f"{tag}{t}{c2}",
                                    name=f"{tag}{t}{c2}")
                    nc.sync.dma_start(
                        out=wt,
                        in_=w_dram[:, idx:idx + 1024].rearrange(
                            "p (j o) -> p j o", j=2))
                    wvt.append(wt)
            for kb in kbs:
                kt2, j = kb // 2, kb % 2
                ks = slice(scol0 + kb * 128, scol0 + (kb + 1) * 128)
                ps = pool_lin.tile([128, 512], F32, tag="ps", name="vps")
                nc.tensor.matmul(ps[:, :], self.ones1x128,
                                 b_row[:, oc * 512:(oc + 1) * 512],
                                 start=True, stop=False)
                for t in range(nps):
                    for c2 in range(ND2):
                        nc.tensor.matmul(ps[:, :], stat_pairs[c2][:, :, ks],
                                         wvt[t * ND2 + c2], start=False,
                                         stop=(t == nps - 1 and c2 == ND2 - 1),
                                         perf_mode=DR)
                sc = vmask_sb[:, kb:kb + 1] if vmask_sb is not None else VS * DS
                nc.vector.tensor_scalar(
                    out=Vt2[kt2][:, j, oc * 8:(oc + 1) * 8, 0:64],
                    in0=ps.rearrange("p (a b) -> p a b", b=64),
                    scalar1=sc, scalar2=None, op0=ALU.mult)

    def attention(self, pools, QT, KT, Vt2, YT2, bias2_sb, bias_base, cmask2_sb,
                  nkb, half0_max_kb):
        """Pairwise attention. QT/KT: ND bf16 tiles. Vt2: fp8 pair tiles
        [128, 2, H, 65] (col 64 = VS ones). YT2: ND2 fp8 pair tiles.
        bias2_sb[:, bias_base + kt2] = per-pair exp bias col.
        cmask2_sb: [128, 4, 2, 512] fp8 stair masks or None.
        half0_max_kb: #key blocks computed for query-half 0 (nkb for cross)."""
        nc = self.nc
        pool_lin, pool_s, ppool, rpool, bcpool = pools
        nkb2 = (nkb + 1) // 2
        # diag slots: (half, kt2) -> cmask slot
        dslots = {(0, 4): 0, (0, 5): 1, (1, 6): 2, (1, 7): 3} \
            if cmask2_sb is not None else {}
        for h in range(H):
            dt, r0 = h // 2, (h % 2) * 64
            y_ps = [pool_lin.tile([65, 512], F32, tag="ps", name="yps")
                    for _ in range(2)]
            for kt2 in range(nkb2):
                js = (0, 1) if 2 * kt2 + 1 < nkb else (0,)
                for half in range(2):
                    if half == 0 and 2 * kt2 >= half0_max_kb:
                        continue
                    qs = slice(half * 512, (half + 1) * 512)
                    s2 = pool_s.tile([128, 2, 512], F32, tag="S2", name="sps")
                    for j in js:
                        kb = 2 * kt2 + j
                        nc.tensor.matmul(
                            s2[:, j, :],
                            KT[dt][r0:r0 + 64, kb * 128:(kb + 1) * 128],
                            QT[dt][r0:r0 + 64, qs], start=True, stop=True)
                    p2 = ppool.tile([128, 2, 512], FP8, tag="P", name="p2")
                    jn = len(js)
                    nc.scalar.activation(
                        out=p2[:, 0:jn, :], in_=s2[:, 0:jn, :], func=AF.Exp,
                        scale=0.125,
                        bias=bias2_sb[:, bias_base + kt2:bias_base + kt2 + 1])
                    slot = dslots.get((half, kt2))
                    if slot is not None:
                        nc.gpsimd.tensor_tensor(out=p2, in0=p2,
                                                in1=cmask2_sb[:, slot],
                                                op=ALU.mult)
                    last_kt2 = (min(nkb, half0_max_kb) + 1) // 2 - 1 \
                        if half == 0 else nkb2 - 1
                    if jn == 2:
                        nc.tensor.matmul(y_ps[half][:, :], Vt2[kt2][:, :, h, :],
                                         p2, start=(kt2 == 0),
                                         stop=(kt2 == last_kt2), perf_mode=DR)
                    else:
                        nc.tensor.matmul(y_ps[half][:, :], Vt2[kt2][:, 0, h, :],
                                         p2[:, 0, :], start=(kt2 == 0),
                                         stop=(kt2 == last_kt2))
            r_t = rpool.tile([65, 1024], BF16, tag="r", name="rt")
            bc_sb = bcpool.tile([64, 1024], BF16, tag="bc", name="bcsb")
            for half in range(2):
                qs = slice(half * 512, (half + 1) * 512)
                nc.vector.reciprocal(out=r_t[64:65, qs], in_=y_ps[half][64:65, :])
                bc_ps = pool_lin.tile([65, 512], F32, tag="ps", name="bcps")
                nc.tensor.matmul(bc_ps[0:64, :], self.ones65[64:65, 0:64],
                                 r_t[64:65, qs], start=True, stop=True)
                nc.vector.tensor_copy(out=bc_sb[:, qs], in_=bc_ps[0:64, :])
                # YT2 = XS * y  (ones col is VS, y_ps rows are VS*y*sumP)
                nc.vector.tensor_tensor(
                    out=YT2[dt // 2][r0:r0 + 64, dt % 2, qs],
                    in0=y_ps[half][:64, :], in1=bc_sb[:, qs], op=ALU.mult)


def build(stage="full", trivial_ln=False):
    return fixup_waits(_build(stage, trivial_ln))


def _build(stage="full", trivial_ln=False):
    nc = bass.Bass()
    def din(name, shape, dt=FP8):
        return nc.dram_tensor(name, shape, dt, kind="ExternalInput")
    xT = din("xT", [D, T], F32)
    xTb = din("xTb", [D, T], BF16)
    memp = din("memp", [128, ND2 * 2 * M])                  # fp8 pairs, x16
    wk = din("wk", [128, 8 * nps_of(PREC['qkv']) * ND2 * 2 * 128])
    wq = din("wq", [128, 8 * nps_of(PREC['qkv']) * ND2 * 2 * 128])
    wv = din("wv", [128, 2 * nps_of(PREC['qkv']) * ND2 * 2 * 512])
    wsap = din("wsap", [128, 8 * nps_of(PREC['sap']) * ND2 * 2 * 128])
    wcaq = din("wcaq", [128, 8 * nps_of(PREC['caq']) * ND2 * 2 * 128])
    wcak = din("wcak", [128, 8 * nps_of(PREC['cakv']) * ND2 * 2 * 128])
    wcav = din("wcav", [128, 2 * nps_of(PREC['cakv']) * ND2 * 2 * 512])
    wcap = din("wcap", [128, 8 * nps_of(PREC['cap']) * ND2 * 2 * 128])
    wff1 = din("wff1", [128, 32 * nps_of(PREC['ff1']) * ND2 * 2 * 128])
    wff2 = din("wff2", [128, 8 * nps_of(PREC['ff2']) * NF2 * 2 * 128])
    qkb_cols = din("qkb_cols", [128, 16], F32)   # Q 0-7, K 8-15 (raw bias)
    cab_cols = din("cab_cols", [128, 16], F32)
    ff1b_cols = din("ff1b_cols", [128, 32], F32)
    bv_row = din("bv_row", [1, D], BF16)         # x WS*XS
    bcav_row = din("bcav_row", [1, D], BF16)     # x WS*XS
    bsap_row = din("bsap_row", [1, D], BF16)     # x WS*XS
    bcap_row = din("bcap_row", [1, D], BF16)     # x WS*XS
    bff2_row = din("bff2_row", [1, D], BF16)     # x WS*XS
    sbias2 = din("sbias2", [128, 16], F32)       # pair bias; cols 8+ = zeros
    vmask = din("vmask", [128, NKC], F32)        # cross (VS*DS or 0) col
    vones = din("vones", [128, NKC * H])         # fp8 VS/0 per (key, kt, h)
    cmask2 = din("cmask2", [128, 4, 2, 512])     # fp8 stair pairs
    ln_rows = {n: din(n, [1, D], BF16) for n in
               ["ln1_w", "ln1_b", "lnm_w", "lnm_b", "ln2_w", "ln2_b"]}
    out = nc.dram_tensor("out", [D, TQ], F32, kind="ExternalOutput")
    dbg = {}
    def dout(name, shape, dt=F32):
        dbg[name] = nc.dram_tensor(name, shape, dt, kind="ExternalOutput")
        return dbg[name]

    with tile.TileContext(nc) as tc, \
         nc.allow_low_precision(reason="fp8/bf16 compute dtype by design"):
        kb_ = KB(nc, tc)
        import contextlib
        est = contextlib.ExitStack()
        with est:
            cp = est.enter_context(tc.tile_pool(name="const", bufs=1))
            pool_lin = est.enter_context(tc.tile_pool(name="plin", bufs=4, space="PSUM"))
            pool_s = est.enter_context(tc.tile_pool(name="ps2", bufs=2, space="PSUM"))
            resid = est.enter_context(tc.tile_pool(name="resid", bufs=1))
            ln_tmp = est.enter_context(tc.tile_pool(name="lntmp", bufs=2))
            strm = est.enter_context(tc.tile_pool(name="strm", bufs=2))
            kb_.strm = strm

            ones128_bf = cp.tile([128, 1], BF16, tag="o128", name="o128")
            nc.vector.memset(ones128_bf, 1.0)
            ones512 = cp.tile([1, 512], BF16, tag="o512", name="o512")
            nc.vector.memset(ones512, 1.0)
            ones1x128 = cp.tile([1, 128], BF16, tag="o1x128", name="o1x128")
            nc.vector.memset(ones1x128, 1.0)
            negXS = cp.tile([1, 512], BF16, tag="no512", name="no512")
            nc.vector.memset(negXS, -XS)
            ones65 = cp.tile([65, 128], BF16, tag="o65", name="o65")
            nc.vector.memset(ones65, 1.0)
            eps_t = cp.tile([1, 1], F32, tag="eps", name="eps")
            nc.vector.memset(eps_t, 1e-5)
            kb_.ones128_bf, kb_.ones512, kb_.ones1x128 = ones128_bf, ones512, ones1x128
            kb_.negXS, kb_.ones65, kb_.eps_t = negXS, ones65, eps_t

            sbias_sb = cp.tile([128, 16], F32, tag="sbias", name="sbias")
            nc.gpsimd.dma_start(out=sbias_sb, in_=sbias2[:, :])
            vmask_sb = cp.tile([128, NKC], F32, tag="vmask", name="vmask")
            nc.gpsimd.dma_start(out=vmask_sb, in_=vmask[:, :])
            qkb_sb = cp.tile([128, 16], F32, tag="qkb", name="qkb")
            nc.gpsimd.dma_start(out=qkb_sb, in_=qkb_cols[:, :])
            cab_sb = cp.tile([128, 16], F32, tag="cab", name="cab")
            nc.gpsimd.dma_start(out=cab_sb, in_=cab_cols[:, :])
            ff1b_sb = cp.tile([128, 32], F32, tag="ff1b", name="ff1b")
            nc.gpsimd.dma_start(out=ff1b_sb, in_=ff1b_cols[:, :])
            brow_dram = {"bv_row": bv_row, "bcav_row": bcav_row,
                         "bsap_row": bsap_row, "bcap_row": bcap_row,
                         "bff2_row": bff2_row}
            brow_pool = est.enter_context(tc.tile_pool(name="brow", bufs=1))
            def brow(n):
                t = brow_pool.tile([1, D], BF16, tag="brow", name="brow")
                nc.gpsimd.dma_start(out=t, in_=brow_dram[n][:, :])
                return t
            lnr = ln_rows

            lnpools = (pool_lin, pool_s, ln_tmp, strm)

            def ts_consumer(dest, dcol0, bias_sb, bias0, dscale):
                def f(ob, tci, ps):
                    nc.vector.tensor_scalar(
                        out=dest[ob][:, dcol0 + tci * 512: dcol0 + tci * 512 + ps.shape[-1]],
                        in0=ps, scalar1=dscale,
                        scalar2=bias_sb[:, bias0 + ob:bias0 + ob + 1],
                        op0=ALU.mult, op1=ALU.add)
                return f

            # ---------------- phase 1: LN1 + QKV + V ----------------
            with tc.tile_pool(name="io_self", bufs=1) as io_self:
                QT = [io_self.tile([128, TQ], BF16, tag=f"QT{i}", name=f"QT{i}")
                      for i in range(ND)]
                KT = [io_self.tile([128, T], BF16, tag=f"KT{i}", name=f"KT{i}")
                      for i in range(ND)]
                Vt2 = [io_self.tile([128, 2, H, 65], FP8, tag=f"V{i}", name=f"V{i}")
                       for i in range(NKB2)]
                YT2 = [io_self.tile([128, 2, TQ], FP8, tag=f"YT{i}", name=f"YT{i}")
                       for i in range(ND2)]
                h1p = [resid.tile([128, 2, T], FP8, tag=f"h1{i}", name=f"h1p{i}")
                       for i in range(ND2)]
                kb_.layernorm(lnpools, ("dram", xTb, 0), TQ,
                              lnr["ln1_w"], lnr["ln1_b"], h1p, 0, trivial=trivial_ln)
                kb_.layernorm(lnpools, ("dram", xTb, TQ), TQ,
                              lnr["ln1_w"], lnr["ln1_b"], h1p, TQ, trivial=trivial_ln)
                if stage == "ln1":
                    o = dout("dbg_h1p", [D, T])
                    for d2 in range(ND2):
                        for j in range(2):
                            tmp = strm.tile([128, T], F32, tag="dbgc", name="dbgc", bufs=1)
                            nc.vector.tensor_copy(out=tmp, in_=h1p[d2][:, j, :])
                            nc.sync.dma_start(
                                out=o[(2*d2+j)*128:(2*d2+j+1)*128, :], in_=tmp)
                npq = nps_of(PREC['qkv'])
                with tc.tile_pool(name="wqk", bufs=3) as wqk:
                    kb_.linear(pool_lin, wqk, wk, ND, h1p, 0, T,
                               ts_consumer(KT, 0, qkb_sb, 8, DS), nps=npq,
                               strip_tag="w")
                    kb_.linear(pool_lin, wqk, wq, ND, h1p, TQ, TQ,
                               ts_consumer(QT, 0, qkb_sb, 0, DS), nps=npq,
                               strip_tag="w")
                with tc.tile_pool(name="wv", bufs=1) as wvp:
                    kb_.vproj(pool_lin, wvp, wv, brow("bv_row"), h1p, 0, Vt2,
                              list(range(NKB)), nps=npq, tag="wv")
                for kt2 in range(NKB2):
                    nc.vector.memset(Vt2[kt2][:, :, :, 64:65], 1.0)
                if stage == "qkv":
                    oq = dout("dbg_QT", [D, TQ]); ok = dout("dbg_KT", [D, T])
                    ov = dout("dbg_V", [NKB * 128, H * 65])
                    for dt in range(ND):
                        tq = strm.tile([128, TQ], F32, tag="dbgc", name="dbgc", bufs=1)
                        nc.vector.tensor_copy(out=tq, in_=QT[dt])
                        nc.sync.dma_start(out=oq[dt*128:(dt+1)*128, :], in_=tq)
                        tk = strm.tile([128, T], F32, tag="dbgc2", name="dbgc2", bufs=1)
                        nc.vector.tensor_copy(out=tk, in_=KT[dt])
                        nc.sync.dma_start(out=ok[dt*128:(dt+1)*128, :], in_=tk)
                    for kt2 in range(NKB2):
                        for j in range(2):
                            tv = strm.tile([128, H * 65], F32, tag="dbgc", name="dbgc", bufs=1)
                            nc.vector.tensor_copy(
                                out=tv, in_=Vt2[kt2][:, j].rearrange("p a b -> p (a b)"))
                            nc.sync.dma_start(out=ov[(2*kt2+j)*128:(2*kt2+j+1)*128, :],
                                              in_=tv)
                # ---------------- phase 2: self attention ----------------
                with tc.tile_pool(name="pcm", bufs=1) as pcm, \
                     tc.tile_pool(name="pp", bufs=3) as ppool, \
                     tc.tile_pool(name="pr", bufs=1) as rpool, \
                     tc.tile_pool(name="pbc", bufs=2) as bcpool:
                    cm_sb = pcm.tile([128, 4, 2, 512], FP8, tag="cm", name="cm")
                    nc.gpsimd.dma_start(out=cm_sb, in_=cmask2[:, :, :, :])
                    kb_.attention((pool_lin, pool_s, ppool, rpool, bcpool),
                                  QT, KT, Vt2, YT2, sbias_sb, 0, cm_sb,
                                  NKB, half0_max_kb=12)
                if stage == "self":
                    o = dout("dbg_YT", [D, TQ])
                    for d2 in range(ND2):
                        for j in range(2):
                            ty = strm.tile([128, TQ], F32, tag="dbgc", name="dbgc", bufs=1)
                            nc.vector.tensor_copy(out=ty, in_=YT2[d2][:, j, :])
                            nc.sync.dma_start(out=o[(2*d2+j)*128:(2*d2+j+1)*128, :],
                                              in_=ty)
                # ---------------- phase 3: sa_proj + residual ----------------
                out1T = [resid.tile([128, TQ], F32, tag=f"o1{i}", name=f"out1T{i}")
                         for i in range(ND)]
                bsap = brow("bsap_row")
                def sap_consumer(ob, tci, ps):
                    cs = slice(tci * 512, (tci + 1) * 512)
                    rv = strm.tile([128, 512], F32, tag="rsd", name="rsd")
                    nc.sync.dma_start(out=rv, in_=xT[ob*128:(ob+1)*128,
                                                     TQ + tci*512: TQ + (tci+1)*512])
                    nc.vector.scalar_tensor_tensor(
                        out=out1T[ob][:, cs], in0=ps, scalar=DS, in1=rv,
                        op0=ALU.mult, op1=ALU.add)
                with tc.tile_pool(name="wsp", bufs=3) as wsp:
                    kb_.linear(pool_lin, wsp, wsap, ND, YT2, 0, TQ, sap_consumer,
                               nps=nps_of(PREC['sap']), bias_row=bsap,
                               strip_tag="w")
                if stage == "out1":
                    o = dout("dbg_out1", [D, TQ])
                    for dt in range(ND):
                        nc.sync.dma_start(out=o[dt*128:(dt+1)*128, :], in_=out1T[dt])
                if stage in ("ln1", "qkv", "self", "out1"):
                    with tc.tile_pool(name="zz", bufs=1) as zz:
                        z = zz.tile([128, TQ], F32, tag="zf", name="zf")
                        nc.vector.memset(z, 0.0)
                        for dt in range(ND):
                            nc.sync.dma_start(out=out[dt*128:(dt+1)*128, :], in_=z)
                    return nc
                # ---------------- phase 4: cross attention ----------------
                KcT = [io_self.tile([128, M], BF16, tag=f"Kc{i}", name=f"Kc{i}")
                       for i in range(ND)]
                Vc2 = [io_self.tile([128, 2, H, 65], FP8, tag=f"Vc{i}", name=f"Vc{i}")
                       for i in range(NKC2 + 1)]
                npc = nps_of(PREC['cakv'])
                with tc.tile_pool(name="pmem", bufs=1) as pmem:
                    memh = [pmem.tile([128, 2, M], FP8, tag=f"m{i}", name=f"memh{i}")
                            for i in range(ND2)]
                    for c2 in range(ND2):
                        nc.sync.dma_start(
                            out=memh[c2],
                            in_=memp[:, c2*2*M:(c2+1)*2*M].rearrange(
                                "p (j m) -> p j m", j=2))
                    with tc.tile_pool(name="wc", bufs=3) as wc:
                        kb_.linear(pool_lin, wc, wcak, ND, memh, 0, M,
                                   ts_consumer(KcT, 0, cab_sb, 8, DS), nps=npc,
                                   strip_tag="w")
                    with tc.tile_pool(name="wvc", bufs=1) as wvc:
                        kb_.vproj(pool_lin, wvc, wcav, brow("bcav_row"), memh, 0,
                                  Vc2, list(range(NKC)), nps=npc,
                                  vmask_sb=vmask_sb, tag="wvc")
                for kb in range(NKC):
                    kt2, j = kb // 2, kb % 2
                    nc.sync.dma_start(
                        out=Vc2[kt2][:, j, :, 64:65],
                        in_=vones[:, kb*H:(kb+1)*H].rearrange("p (a b) -> p a b", b=1))
                QcT = [io_self.tile([128, TQ], BF16, tag=f"QT{i}", name=f"Qc{i}")
                       for i in range(ND)]
                YcT2 = [io_self.tile([128, 2, TQ], FP8, tag=f"YT{i}", name=f"Yc{i}")
                        for i in range(ND2)]
                with tc.tile_pool(name="ph2", bufs=1) as ph2, \
                     tc.tile_pool(name="wc2", bufs=3) as wc2:
                    h2p = [ph2.tile([128, 2, TQ], FP8, tag=f"h2{i}", name=f"h2{i}")
                           for i in range(ND2)]
                    kb_.layernorm(lnpools, ("sbuf", out1T, 0), TQ,
                                  lnr["lnm_w"], lnr["lnm_b"], h2p, 0,
                                  trivial=trivial_ln)
                    kb_.linear(pool_lin, wc2, wcaq, ND, h2p, 0, TQ,
                               ts_consumer(QcT, 0, cab_sb, 0, DS),
                               nps=nps_of(PREC['caq']), strip_tag="w")
                with tc.tile_pool(name="pp2", bufs=2) as ppool, \
                     tc.tile_pool(name="pr2", bufs=2) as rpool, \
                     tc.tile_pool(name="pbc2", bufs=2) as bcpool:
                    kb_.attention((pool_lin, pool_s, ppool, rpool, bcpool),
                                  QcT, KcT, Vc2, YcT2, sbias_sb, 8, None,
                                  NKC, half0_max_kb=NKC)
                out2T = out1T
                bcap = brow("bcap_row")
                def cap_consumer(ob, tci, ps):
                    cs = slice(tci * 512, (tci + 1) * 512)
                    nc.vector.scalar_tensor_tensor(
                        out=out2T[ob][:, cs], in0=ps, scalar=DS,
                        in1=out1T[ob][:, cs], op0=ALU.mult, op1=ALU.add)
                with tc.tile_pool(name="wcp", bufs=3) as wcp:
                    kb_.linear(pool_lin, wcp, wcap, ND, YcT2, 0, TQ, cap_consumer,
                               nps=nps_of(PREC['cap']), bias_row=bcap,
                               strip_tag="w")
            if stage == "out2":
                o = dout("dbg_out2", [D, TQ])
                for dt in range(ND):
                    nc.sync.dma_start(out=o[dt*128:(dt+1)*128, :], in_=out2T[dt])
                with tc.tile_pool(name="zz", bufs=1) as zz:
                    z = zz.tile([128, TQ], F32, tag="zf", name="zf")
                    nc.vector.memset(z, 0.0)
                    for dt in range(ND):
                        nc.sync.dma_start(out=out[dt*128:(dt+1)*128, :], in_=z)
                return nc

            # ---------------- phase 5: FFN ----------------
            with tc.tile_pool(name="io_ffn", bufs=1) as io_f:
                GT = [io_f.tile([128, 2, TQ], FP8, tag=f"G{i}", name=f"G{i}")
                      for i in range(NF2)]
                def ff1_consumer(ob, tci, ps):
                    cs = slice(tci * 512, (tci + 1) * 512)
                    nc.scalar.activation(out=GT[ob // 2][:, ob % 2, cs],
                                         in_=ps, func=AF.Gelu, scale=DS,
                                         bias=ff1b_sb[:, ob:ob + 1])
                with tc.tile_pool(name="ph3", bufs=1) as ph3, \
                     tc.tile_pool(name="wf1", bufs=3) as wf1:
                    h3p = [ph3.tile([128, 2, TQ], FP8, tag=f"h3{i}", name=f"h3{i}")
                           for i in range(ND2)]
                    kb_.layernorm(lnpools, ("sbuf", out2T, 0), TQ,
                                  lnr["ln2_w"], lnr["ln2_b"], h3p, 0,
                                  trivial=trivial_ln)
                    kb_.linear(pool_lin, wf1, wff1, NF, h3p, 0, TQ, ff1_consumer,
                               nps=nps_of(PREC['ff1']), strip_tag="w")
                bff2 = brow("bff2_row")
                with tc.tile_pool(name="wf2", bufs=2) as wf2, \
                     tc.tile_pool(name="oo", bufs=3) as oo:
                    ots = {}
                    def ff2_consumer(ob, tci, ps):
                        cs = slice(tci * 512, (tci + 1) * 512)
                        if ob not in ots:
                            ots[ob] = oo.tile([128, TQ], F32, tag="ot", name="ot")
                        nc.vector.scalar_tensor_tensor(
                            out=ots[ob][:, cs], in0=ps, scalar=DS * XS,
                            in1=out2T[ob][:, cs], op0=ALU.mult, op1=ALU.add)
                        if tci == 1:
                            nc.sync.dma_start(out=out[ob*128:(ob+1)*128, :],
                                              in_=ots[ob])
                    kb_.linear(pool_lin, wf2, wff2, ND, GT, 0, TQ, ff2_consumer,
                               nps=nps_of(PREC['ff2']), nc2=NF2,
                               bias_row=bff2, strip_tag="wf2")
    return nc


# ---- scheduler-sim makespan probe -------------------------------------------
SIM_TIME = [0]
def _install_sim_probe():
    import concourse.tile as _t
    import concourse.bass_interp as _bi
    if getattr(_t, "_sim_probe", False):
        return
    _t._sim_probe = True
    orig = _bi.CoreSim.simulate
    def simulate(self, *a, **k):
        r = orig(self, *a, **k)
        try:
            SIM_TIME[0] = max(SIM_TIME[0], int(self.time))
        except Exception:
            pass
        return r
    _bi.CoreSim.simulate = simulate
_install_sim_probe()


import numpy as np
import ml_dtypes
from concourse.bass_utils import run_bass_kernel_spmd
BF = ml_dtypes.bfloat16
F8 = ml_dtypes.float8_e4m3
M_ORIG = 2048
NEG = -10000.0


def _stat_pack(w, nps):
    """w [K, N] f32 -> [128, (N/128)*nps*(K/256)*2*128] fp8 (terms at scale WS)."""
    K, N = w.shape
    nc2, OB = K // 256, N // 128
    w = w.astype(np.float32)
    terms = []
    a = np.clip(w * WS, -240, 240).astype(F8)
    terms.append(a)
    r = w - a.astype(np.float32) / WS
    if nps == 2:
        terms.append(np.clip(r * WS, -240, 240).astype(F8))
    outp = np.zeros((128, OB, nps, nc2, 2, 128), F8)
    for t, tm in enumerate(terms[:nps]):
        v = tm.reshape(nc2, 2, 128, OB, 128)
        outp[:, :, t] = v.transpose(2, 3, 0, 1, 4)
    return np.ascontiguousarray(outp.reshape(128, -1))


def _mov_pack(w, nps):
    """w [K, 1024] f32 -> [128, 2*nps*(K/256)*2*512] fp8 moving layout."""
    K, N = w.shape
    assert N == 1024
    nc2 = K // 256
    w = w.astype(np.float32)
    terms = [np.clip(w * WS, -240, 240).astype(F8)]
    if nps == 2:
        r = w - terms[0].astype(np.float32) / WS
        terms.append(np.clip(r * WS, -240, 240).astype(F8))
    outp = np.zeros((128, 2, nps, nc2, 2, 512), F8)
    for t, tm in enumerate(terms):
        v = tm.reshape(nc2, 2, 128, 2, 512)      # [c2, j, p, oc, col]
        outp[:, :, t] = v.transpose(2, 3, 0, 1, 4)
    return np.ascontiguousarray(outp.reshape(128, -1))


def _cols(vec, nb):
    return np.ascontiguousarray(vec.reshape(nb, 128).T).astype(np.float32)


def _stair2(kb2, role):
    """[128, 2, 512] stair for pair kb2, role-specific geometry."""
    keyoff = 1024 if role == 0 else 0
    qoff = 0 if role == 0 else 1024
    half = 0 if kb2 < 6 else 1
    m = np.zeros((128, 2, 512), np.float32)
    for j in range(2):
        key = (2 * kb2 + j) * 128 - keyoff + np.arange(128)[:, None]
        q = qoff + half * 512 + np.arange(512)[None, :]
        m[:, j, :] = (key <= q)
    return m


def prep_inputs(inputs):
    g = {k: np.asarray(v) for k, v in inputs.items()}
    np1 = nps_of(PREC['qkv']); np2 = nps_of(PREC['sap'])
    np3 = nps_of(PREC['caq']); np4 = nps_of(PREC['cakv'])
    np5 = nps_of(PREC['cap']); np6 = nps_of(PREC['ff1'])
    np7 = nps_of(PREC['ff2'])
    sc = WS * XS
    shared = {
        "wq": _stat_pack(g["sa_qkv_w"][:, 0:1024], np1),
        "wk": _stat_pack(g["sa_qkv_w"][:, 1024:2048], np1),
        "wv": _mov_pack(g["sa_qkv_w"][:, 2048:3072], np1),
        "wsap": _stat_pack(g["sa_proj_w"], np2),
        "wcaq": _stat_pack(g["ca_q_w"], np3),
        "wcak": _stat_pack(g["ca_kv_w"][:, 0:1024], np4),
        "wcav": _mov_pack(g["ca_kv_w"][:, 1024:2048], np4),
        "wcap": _stat_pack(g["ca_proj_w"], np5),
        "wff1": _stat_pack(g["ff1_w"], np6),
        "wff2": _stat_pack(g["ff2_w"], np7),
        "bv_row": (g["sa_qkv_b"][2048:3072] * sc).reshape(1, -1).astype(BF),
        "bcav_row": (g["ca_kv_b"][1024:2048] * sc).reshape(1, -1).astype(BF),
        "bsap_row": (g["sa_proj_b"] * sc).reshape(1, -1).astype(BF),
        "bcap_row": (g["ca_proj_b"] * sc).reshape(1, -1).astype(BF),
        "bff2_row": (g["ff2_b"] * sc).reshape(1, -1).astype(BF),
        "qkb_cols": np.concatenate([_cols(g["sa_qkv_b"][0:1024], 8),
                                    _cols(g["sa_qkv_b"][1024:2048], 8)], axis=1),
        "cab_cols": np.concatenate([_cols(g["ca_q_b"], 8),
                                    _cols(g["ca_kv_b"][0:1024], 8)], axis=1),
        "ff1b_cols": _cols(g["ff1_b"], 32),
        "ln1_w": g["ln1_w"].reshape(1, -1).astype(BF),
        "ln1_b": g["ln1_b"].reshape(1, -1).astype(BF),
        "lnm_w": g["lnm_w"].reshape(1, -1).astype(BF),
        "lnm_b": g["lnm_b"].reshape(1, -1).astype(BF),
        "ln2_w": g["ln2_w"].reshape(1, -1).astype(BF),
        "ln2_b": g["ln2_b"].reshape(1, -1).astype(BF),
    }
    cmask_by_role = []
    for role in range(2):
        cm = np.zeros((128, 4, 2, 512), np.float32)
        for slot, kb2 in enumerate((4, 5, 6, 7)):
            cm[:, slot] = _stair2(kb2, role)
        cmask_by_role.append(cm.astype(F8))
    sbias_by_role = [np.zeros((128, 16), np.float32) for _ in range(2)]
    sbias_by_role[0][:, 0:4] = NEG
    in_maps = []
    for core in range(8):
        b, role = core // 2, core % 2
        x = np.asarray(g["x"][b], np.float32)
        if role == 0:
            xt = np.concatenate([x[0:1024].T, x[0:1024].T], axis=1)
        else:
            xt = x.T
        mask = np.asarray(g["mem_mask"][b] != 0)
        order = np.argsort(~mask, kind="stable")[:M]
        memc = np.asarray(g["mem"][b], np.float32)[order]      # [MC, D]
        mvalid = mask[order]
        # fp8 pair layout [p, c2, j, m], x XS
        mp = np.clip(memc.T * XS, -240, 240).astype(F8).reshape(ND2, 2, 128, M)
        mp = np.ascontiguousarray(mp.transpose(2, 0, 1, 3).reshape(128, -1))
        vmask_cols = np.where(mvalid, VS / sc, 0.0).astype(np.float32)
        vones_f = np.where(mvalid, 1.0, 0.0).astype(np.float32)
        im = dict(shared)
        im.update({
            "xT": np.ascontiguousarray(xt, dtype=np.float32),
            "xTb": np.ascontiguousarray(xt).astype(BF),
            "memp": mp,
            "vmask": np.ascontiguousarray(
                vmask_cols.reshape(NKC, 128).T).astype(np.float32),
            "vones": np.ascontiguousarray(
                np.repeat(vones_f.reshape(NKC, 128).T[:, :, None], H, axis=2)
                .reshape(128, -1)).astype(F8),
            "sbias2": sbias_by_role[role],
            "cmask2": cmask_by_role[role],
        })
        in_maps.append(im)
    return in_maps


def gather(results):
    out = np.zeros((4, 2048, 1024), np.float32)
    for core in range(8):
        b, role = core // 2, core % 2
        out[b, role * 1024:(role + 1) * 1024, :] = results[core]["out"].T
    return out


_NC = None

def kernel(**inputs):
    """Full decoder block on 8 NeuronCores: batch x query-half data parallel,
    fp8 DoubleRow matmuls with weight compensation, fp32 residual stream."""
    global _NC
    if _NC is None:
        trivial = all(
            np.all(np.asarray(inputs[w]) == 1.0) and np.all(np.asarray(inputs[b]) == 0.0)
            for w, b in [("ln1_w", "ln1_b"), ("lnm_w", "lnm_b"), ("ln2_w", "ln2_b")])
        _NC = build(stage="full", trivial_ln=trivial)
    in_maps = prep_inputs(inputs)
    res = run_bass_kernel_spmd(_NC, in_maps, core_ids=list(range(8)))
    return gather(res.results)

